# revision 3
# baseline (speedup 1.0000x reference)
"""AttentionLoss (BCE + dice over FPN attention maps) on 8 TRN2 NeuronCores.

Sharding: data-parallel over batch B=16 -> 2 images per core.

Device algorithm per (image b, level l):
  - Build per-box row/col interval indicators from host-prepped bounds:
       row[n,h] = (h > alo[n]) & (h < ahi[n])   (sel folded into ahi)
       col[n,w] = (w > clo[n]) & (w < chi[n])
  - Rasterize union-of-boxes mask counts on TensorE: cnt = row^T @ col.
  - Threshold on VectorE:  g' = (cnt<=0) - 0.5  in {+0.5 empty, -0.5 covered}
    (accum -> Sum g' = N/2 - Sm).
  - Per channel c (fused, one DVE + one ACT op per element):
       e' = (p - 0.5) * g'            (scalar_tensor_tensor, accum -> Se)
       lnq = Ln(-2*e' + 0.5)          (= log p where mask=1, log(1-p) where 0;
                                        activation accum -> Sum ln q)
  - All accumulator columns land in a [128, NCOL] stats tile, DMA'd out.
Host: tiny closed-form combine of the per-(b,l,c) sums into bce+dice means.
"""

import os
import sys
from contextlib import ExitStack

import numpy as np

sys.path.insert(0, "/opt/trn_rl_repo")

LEVEL_SIZES = [256, 128, 64, 32, 16]
B, N, C = 16, 64, 8
NCORES = 8
IMGS_PER_CORE = B // NCORES
EPS = 1e-8

# stats column layout (per core): for each (b in 0..1, l in 0..4, chunk):
#   e-cols:  one col per (b, l, chunk, c)
#   b-cols:  one col per (b, l, chunk, c)   (sum of Ln q)
#   g-cols:  one col per (b, l, chunk)      (sum of g')
_CHUNKS = [2, 1, 1, 1, 1]  # 128-row chunks per level


def _col_layout():
    # DVE-written tile (stats_v): g-cols then e-cols.  ACT tile (stats_a): b-cols.
    e_cols, b_cols, g_cols = {}, {}, {}
    kv = ka = 0
    for b in range(IMGS_PER_CORE):
        for l in range(5):
            for ch in range(_CHUNKS[l]):
                g_cols[(b, l, ch)] = kv
                kv += 1
                for c in range(C):
                    e_cols[(b, l, ch, c)] = kv
                    kv += 1
                    b_cols[(b, l, ch, c)] = ka
                    ka += 1
    return e_cols, b_cols, g_cols, kv, ka


E_COLS, B_COLS, G_COLS, NCOLV, NCOLA = _col_layout()

_PROGRAM_CACHE = {}
LAST_RESULTS = None


def _build_program():
    import concourse.bass as bass
    import concourse.bacc as bacc
    import concourse.mybir as mybir
    import concourse.tile as tile

    f32 = mybir.dt.float32
    i32 = mybir.dt.int32
    Alu = mybir.AluOpType
    Act = mybir.ActivationFunctionType

    nc = bacc.Bacc(name="attnloss")

    att = [
        nc.declare_dram_parameter(f"attn{l}", [IMGS_PER_CORE, C, s, s], f32, False)
        for l, s in enumerate(LEVEL_SIZES)
    ]
    # bounds[:, l*4 + {0,1,2,3}] = alo, ahi, clo, chi ; partitions = (img, box)
    bounds = nc.declare_dram_parameter("bounds", [128, 20], f32, False)
    stats_v_out = nc.declare_dram_parameter("stats_v", [128, NCOLV], f32, True)
    stats_a_out = nc.declare_dram_parameter("stats_a", [128, NCOLA], f32, True)

    with ExitStack() as ctx:
        tc = ctx.enter_context(tile.TileContext(nc))
        const_p = ctx.enter_context(tc.tile_pool(name="const", bufs=1))
        row_p = ctx.enter_context(tc.tile_pool(name="rows", bufs=3))
        g_p = ctx.enter_context(tc.tile_pool(name="gmask", bufs=3))
        data_p = ctx.enter_context(tc.tile_pool(name="data", bufs=3))
        e_p = ctx.enter_context(tc.tile_pool(name="etile", bufs=3))
        scr_p = ctx.enter_context(tc.tile_pool(name="scr", bufs=2))
        psum_p = ctx.enter_context(tc.tile_pool(name="psum", bufs=3, space="PSUM"))

        # persistent tiles
        stats_v = const_p.tile([128, NCOLV], f32)
        nc.vector.memset(stats_v, 0.0)
        stats_a = const_p.tile([128, NCOLA], f32)
        nc.scalar.memzero(stats_a)
        bnd = const_p.tile([128, 20], f32)
        nc.gpsimd.dma_start(out=bnd, in_=bounds[:, :])
        iota_i = const_p.tile([128, 256], i32)
        nc.gpsimd.iota(iota_i, pattern=[[1, 256]], base=0, channel_multiplier=0)
        iota_f = const_p.tile([128, 256], f32)
        nc.vector.tensor_copy(iota_f, iota_i)
        bias05 = const_p.tile([128, 1], f32)
        nc.vector.memset(bias05, 0.5)

        for l, S in enumerate(LEVEL_SIZES):
            # --- row/col indicators for both images: partitions = (img, box)
            rowA = row_p.tile([128, S], f32, tag="rowA")
            row = row_p.tile([128, S], f32, tag="row")
            colA = row_p.tile([128, S], f32, tag="rowA")
            col = row_p.tile([128, S], f32, tag="row")
            io = iota_f[:, :S]

            def bcast(j):
                return bnd[:, 4 * l + j : 4 * l + j + 1].broadcast_to((128, S))

            nc.vector.tensor_tensor(out=rowA, in0=io, in1=bcast(0), op=Alu.is_gt)
            nc.vector.tensor_tensor(out=row, in0=io, in1=bcast(1), op=Alu.is_lt)
            nc.vector.tensor_tensor(out=row, in0=row, in1=rowA, op=Alu.logical_and)
            nc.vector.tensor_tensor(out=colA, in0=io, in1=bcast(2), op=Alu.is_gt)
            nc.vector.tensor_tensor(out=col, in0=io, in1=bcast(3), op=Alu.is_lt)
            nc.vector.tensor_tensor(out=col, in0=col, in1=colA, op=Alu.logical_and)

            nchunk = _CHUNKS[l]
            hchunk = min(128, S)
            for b in range(IMGS_PER_CORE):
                for ch in range(nchunk):
                    h0 = ch * hchunk
                    # rasterize: cnt[h, w] = sum_n row[n, h0+h] * col[n, w]
                    cnt = psum_p.tile([hchunk, S], f32, tag="cnt")
                    nc.tensor.matmul(
                        out=cnt,
                        lhsT=row[64 * b : 64 * b + 64, h0 : h0 + hchunk],
                        rhs=col[64 * b : 64 * b + 64, :],
                        start=True,
                        stop=True,
                    )
                    # m01 = (cnt > 0) in {1,0}; accum(add) -> Sm directly
                    m01 = g_p.tile([hchunk, S], f32, tag="m01")
                    gcol = G_COLS[(b, l, ch)]
                    nc.vector.tensor_scalar(
                        out=m01, in0=cnt, scalar1=0.0, scalar2=None,
                        op0=Alu.is_gt, op1=Alu.add,
                        accum_out=stats_v[:hchunk, gcol : gcol + 1],
                    )
                    # g = m - 0.5 in {-0.5 empty, +0.5 covered}
                    g = g_p.tile([hchunk, S], f32, tag="g")
                    nc.vector.tensor_scalar(
                        out=g, in0=m01, scalar1=0.5, scalar2=None,
                        op0=Alu.subtract,
                    )
                    # load attention rows chunk for all channels: [h, C, S]
                    p_t = data_p.tile([hchunk, C, S], f32, tag="p")
                    src = att[l][b, :, h0 : h0 + hchunk, :].rearrange("c h w -> h c w")
                    nc.sync.dma_start(out=p_t, in_=src)
                    for c in range(C):
                        e_t = e_p.tile([hchunk, S], f32, tag="e")
                        ecol = E_COLS[(b, l, ch, c)]
                        bcol = B_COLS[(b, l, ch, c)]
                        nc.vector.scalar_tensor_tensor(
                            out=e_t, in0=p_t[:, c, :], scalar=0.5, in1=g,
                            op0=Alu.subtract, op1=Alu.mult,
                            accum_out=stats_v[:hchunk, ecol : ecol + 1],
                        )
                        scr = scr_p.tile([hchunk, S], f32, tag="scr")
                        nc.scalar.activation(
                            out=scr, in_=e_t, func=Act.Ln,
                            bias=bias05[:hchunk, :], scale=2.0,
                            accum_out=stats_a[:hchunk, bcol : bcol + 1],
                        )

        nc.sync.dma_start(out=stats_v_out[:, :], in_=stats_v)
        nc.sync.dma_start(out=stats_a_out[:, :], in_=stats_a)
    nc.compile()
    return nc


def _host_bounds(bboxs, img_h, img_w, alpha, beta):
    """bounds [B, 5, 4, 64] float32 (alo, ahi, clo, chi per level/box)."""
    h = np.float32(img_h)
    w = np.float32(img_w)
    bb = bboxs.astype(np.float32)
    x1, y1, x2, y2 = bb[..., 0], bb[..., 1], bb[..., 2], bb[..., 3]
    valid = (x1 <= w) & (y1 <= h) & (x2 <= w) & (y2 <= h)
    area = np.abs((x2 - x1) * (y2 - y1))
    out = np.empty((B, 5, 4, N), np.float32)
    for l, S in enumerate(LEVEL_SIZES):
        side = np.float32(2.0 ** (l + int(alpha)))
        min_a = side * side
        max_a = (side * np.float32(int(beta))) ** 2
        sel = valid & (area >= min_a) & (area <= max_a)
        sx = np.float32(S) / w
        sy = np.float32(S) / h
        out[:, l, 0] = y1 * sy - np.float32(1.0)
        out[:, l, 1] = np.where(sel, y2 * sy + np.float32(1.0), np.float32(-1e9))
        out[:, l, 2] = x1 * sx - np.float32(1.0)
        out[:, l, 3] = x2 * sx + np.float32(1.0)
    return out, valid


def kernel(**inputs):
    from concourse.bass_utils import run_bass_kernel_spmd

    attns = [inputs[f"attn{l}"] for l in range(5)]
    attns = [np.asarray(a, np.float32) for a in attns]
    bboxs = np.asarray(inputs["bboxs"], np.float32)
    img_h, img_w = int(inputs["img_h"]), int(inputs["img_w"])
    alpha, beta = int(inputs["alpha"]), int(inputs["beta"])

    bounds, valid = _host_bounds(bboxs, img_h, img_w, alpha, beta)

    key = "prog"
    if key not in _PROGRAM_CACHE:
        print("[kernel] building bass program...", flush=True)
        _PROGRAM_CACHE[key] = _build_program()
        print("[kernel] build done", flush=True)
    nc = _PROGRAM_CACHE[key]

    in_maps = []
    for k in range(NCORES):
        b0 = IMGS_PER_CORE * k
        m = {f"attn{l}": np.ascontiguousarray(attns[l][b0 : b0 + IMGS_PER_CORE])
             for l in range(5)}
        # device bounds tile: [128, 20] partitions=(img,box), cols = l*4+j
        bt = np.zeros((128, 20), np.float32)
        for bi in range(IMGS_PER_CORE):
            for l in range(5):
                for j in range(4):
                    bt[64 * bi : 64 * bi + 64, 4 * l + j] = bounds[b0 + bi, l, j]
        m["bounds"] = bt
        in_maps.append(m)

    print("[kernel] launching spmd run...", flush=True)
    res = run_bass_kernel_spmd(nc, in_maps, core_ids=list(range(NCORES)))
    print("[kernel] spmd run done", flush=True)
    global LAST_RESULTS
    LAST_RESULTS = res

    # ---- host combine (tiny): per (b,l,c) closed-form from device sums
    per_image = np.zeros(B, np.float64)
    for k in range(NCORES):
        colsum = res.results[k]["stats_v"].astype(np.float64).sum(axis=0)
        colsum_a = res.results[k]["stats_a"].astype(np.float64).sum(axis=0)
        for bi in range(IMGS_PER_CORE):
            b = IMGS_PER_CORE * k + bi
            acc = 0.0
            for l, S in enumerate(LEVEL_SIZES):
                npix = float(S * S)
                Sm = sum(colsum[G_COLS[(bi, l, ch)]] for ch in range(_CHUNKS[l]))
                for c in range(C):
                    Se = sum(colsum[E_COLS[(bi, l, ch, c)]] for ch in range(_CHUNKS[l]))
                    Sb = sum(colsum_a[B_COLS[(bi, l, ch, c)]] for ch in range(_CHUNKS[l]))
                    Sp = float(attns[l][b, c].astype(np.float64).sum())
                    Spm = Se + 0.5 * Sp + 0.5 * Sm - 0.25 * npix
                    bce = -Sb / npix
                    inter = 2.0 * Spm + EPS
                    union = Sp + Sm + EPS
                    dice = 1.0 - inter / union
                    acc += 0.5 * bce + 0.5 * dice
            per_image[b] = acc / (5 * C)
    has_box = valid.any(axis=1)
    per_image = np.where(has_box, per_image, 0.0)
    return np.asarray([per_image.mean()], np.float32)

